# revision 21
# baseline (speedup 1.0000x reference)
"""MoE layer (top-2 of 8 experts, SwiGLU FFN) on 8 Trainium2 NeuronCores.

Expert-parallel sharding: core e holds expert e's weights (W1/W2/W3 slices).
Host computes the (tiny) router matmul + top-2 dispatch, gathers each
expert's tokens, and ships them transposed so the device kernel is a pure
grouped GEMM:

    h.T = W1e.T @ Xe.T ; g.T = W3e.T @ Xe.T          (contract over D)
    a.T = silu(h.T) * g.T                            (ACT + DVE)
    y.T = W2e.T @ a.T                                (contract over F)

All tensors travel in bfloat16 (PSUM accumulates fp32): this halves HBM
traffic vs fp32, and bf16 weights get Fast-Weight-Load (2 elems/cycle)
so LDWEIGHTS hides fully under the matmul stream.  End-to-end rel err is
~4.5e-3 (vs 2e-2 budget).

Schedule notes (from perfetto/NTFF analysis of the fp32r baseline):
  * Tokens are split into nb=2 column blocks; the k-loop is OUTER and the
    block-loop inner so each stationary weight panel is reused by two
    consecutive matmuls (halves LDWEIGHTS work).
  * x and the f=0 weight panels are DMAed in small chunks, issued in
    exactly the order the first matmuls consume them -- the baseline
    waited ~13us for one monolithic x transfer before the first matmul.
  * act (stage-1 output) is one tile per (f, block) so stage-2's first
    matmul doesn't wait on the whole stage-1 output (writer-granularity
    dependence tracking).
  * W2[0] is prefetched at kernel start; y leaves as bf16.
  * PSUM pools: h 2 bufs + g 3 + y 3 = 8 banks exactly.

Host applies the router probabilities and scatter-adds the per-expert
outputs back into the full [B,S,D] output in fp64.
"""

import numpy as np
import ml_dtypes

import concourse.tile as tile
from concourse import bacc, mybir
from concourse.bass_utils import run_bass_kernel_spmd

N_CORES = 8
P = 128  # SBUF partitions / matmul tile edge
BF16 = ml_dtypes.bfloat16

# Results of the most recent device run (for the test harness / profiling).
last_results = None

_NC_CACHE = {}


def _build_nc(ko, ft, dt, blk):
    """Device program: SwiGLU FFN for one expert over C = 2*blk tokens.

    ko = D/128 (stage-1 contraction tiles), ft = F/128 (stage-1 psum tiles),
    dt = D/128 (stage-2 psum tiles), token columns = 2 blocks of width blk.
    Blocks sit at a 64-byte-aligned stride (bs) inside the x panel: an
    unaligned moving-operand start costs the PE ~4ns per matmul.
    """
    C = 2 * blk
    bs = -(-blk // 32) * 32          # block stride in elements (64B mult)
    Cx = 2 * bs                      # x-panel row stride per k-slab
    f32 = mybir.dt.float32
    bf16 = mybir.dt.bfloat16
    silu = mybir.ActivationFunctionType.Silu
    fg_n = ft // 2  # stage-1 weight panels travel in f-pair groups

    nc = bacc.Bacc("TRN2", target_bir_lowering=False, debug=False,
                   num_devices=N_CORES)
    # DMA throughput here is packet-rate-bound (~0.2 GB/s per byte of
    # contiguous per-partition row), so every DRAM layout below keeps
    # partition rows >= 2 KB contiguous.
    xt_d = nc.dram_tensor("xt", [P, ko * Cx], bf16, kind="ExternalInput")
    w1_d = nc.dram_tensor("w1t", [fg_n, P, 2 * ko * P], bf16,
                          kind="ExternalInput")
    w3_d = nc.dram_tensor("w3t", [fg_n, P, 2 * ko * P], bf16,
                          kind="ExternalInput")
    w2_d = nc.dram_tensor("w2t", [dt, P, ft * P], bf16, kind="ExternalInput")
    yt_d = nc.dram_tensor("yt", [dt, P, C], bf16, kind="ExternalOutput")

    with tile.TileContext(nc) as tc:
        with (
            tc.tile_pool(name="xpool", bufs=1) as xpool,
            tc.tile_pool(name="wpool", bufs=3) as wpool,
            tc.tile_pool(name="w2pool", bufs=3) as w2pool,
            tc.tile_pool(name="actpool", bufs=1) as actpool,
            tc.tile_pool(name="hpool", bufs=3) as hpool,
            tc.tile_pool(name="ypool", bufs=3) as ypool,
            tc.tile_pool(name="psh", bufs=1, space="PSUM") as psh,
            tc.tile_pool(name="psg", bufs=1, space="PSUM") as psg,
            tc.tile_pool(name="psy", bufs=2, space="PSUM") as psy,
        ):
            # ---- startup DMAs, in first-matmul consumption order ----
            x_sb = xpool.tile([P, ko * Cx], bf16, name="x_sb", tag="x")

            def dma_x_ks(k0, k1):
                nc.sync.dma_start(out=x_sb[:, k0 * Cx:k1 * Cx],
                                  in_=xt_d[:, k0 * Cx:k1 * Cx])

            def dma_w_half(sb, dram, fg, half):
                nc.sync.dma_start(
                    out=sb[:, half * ko * P:(half + 1) * ko * P],
                    in_=dram[fg][:, half * ko * P:(half + 1) * ko * P])

            def dma_w_group(dram, fg, tag):
                # one group = panels for f=2*fg and 2*fg+1, loaded as two
                # half-DMAs so f=2*fg's matmuls gate on only its own panel
                sb = wpool.tile([P, 2 * ko * P], bf16, tag=tag,
                                name=f"{tag}_{fg}")
                dma_w_half(sb, dram, fg, 0)
                dma_w_half(sb, dram, fg, 1)
                return sb

            w1g = wpool.tile([P, 2 * ko * P], bf16, tag="w1g", name="w1g_0")
            w3g = wpool.tile([P, 2 * ko * P], bf16, tag="w3g", name="w3g_0")
            # f=0's panel in quarters: the very first matmul gates on only
            # k=0..3 of W1[f=0] plus the first two x k-slabs
            hk = ko * P // 2
            nc.sync.dma_start(out=w1g[:, :hk], in_=w1_d[0][:, :hk])
            dma_x_ks(0, 2)
            nc.sync.dma_start(out=w1g[:, hk:2 * hk], in_=w1_d[0][:, hk:2 * hk])
            dma_x_ks(2, 4)
            dma_x_ks(4, 8)
            dma_w_half(w3g, w3_d, 0, 0)
            dma_w_half(w1g, w1_d, 0, 1)
            dma_w_half(w3g, w3_d, 0, 1)

            # PE warmup: scratch matmuls with no data dependencies, so the
            # HAM clock-gate reaches 8/8 while the startup DMAs are still in
            # flight (PE would otherwise sit idle ~10us and start at 1.2GHz).
            warm_sb = hpool.tile([P, blk], bf16, name="warm", tag="warm")
            nc.vector.memset(warm_sb[:], 0.0)
            warm_ps = psy.tile([P, blk], f32, name="warm_ps", tag="py0")
            for _ in range(8):
                nc.tensor.matmul(warm_ps[:, :P], warm_sb[:, :P],
                                 warm_sb[:, :P], start=True, stop=True)

            act_t = [[None, None] for _ in range(ft)]
            w2_sb0 = None

            # ---- stage 1: h = x@W1, g = x@W3, act = silu(h)*g ----
            for f in range(ft):
                if f > 0 and f % 2 == 0:
                    w1g = dma_w_group(w1_d, f // 2, "w1g")
                    w3g = dma_w_group(w3_d, f // 2, "w3g")
                if f == 4:
                    # stage-2's first weight panel: issued once the startup
                    # x/w burst has drained (issuing it earlier starves the
                    # f=1..2 panel DMAs and stalls the PE ramp)
                    w2_sb0 = w2pool.tile([P, ft * P], bf16, name="w2_0",
                                         tag="w2")
                    nc.sync.dma_start(out=w2_sb0[:], in_=w2_d[0])
                base = (f % 2) * ko * P
                ph0 = psh.tile([P, blk], f32, name="ph0", tag="ph0")
                ph1 = psh.tile([P, blk], f32, name="ph1", tag="ph1")
                for k in range(ko):
                    w = w1g[:, base + k * P:base + (k + 1) * P]
                    nc.tensor.matmul(ph0[:], w, x_sb[:, k * Cx:k * Cx + blk],
                                     start=(k == 0), stop=(k == ko - 1))
                    nc.tensor.matmul(ph1[:], w,
                                     x_sb[:, k * Cx + bs:k * Cx + bs + blk],
                                     start=(k == 0), stop=(k == ko - 1))
                pg0 = psg.tile([P, blk], f32, name="pg0", tag="pg0")
                pg1 = psg.tile([P, blk], f32, name="pg1", tag="pg1")
                for k in range(ko):
                    w = w3g[:, base + k * P:base + (k + 1) * P]
                    nc.tensor.matmul(pg0[:], w, x_sb[:, k * Cx:k * Cx + blk],
                                     start=(k == 0), stop=(k == ko - 1))
                    nc.tensor.matmul(pg1[:], w,
                                     x_sb[:, k * Cx + bs:k * Cx + bs + blk],
                                     start=(k == 0), stop=(k == ko - 1))
                for b, ph, pg in ((0, ph0, pg0), (1, ph1, pg1)):
                    sh = hpool.tile([P, blk], bf16, name=f"sh{b}", tag="sh")
                    nc.scalar.activation(sh[:], ph[:], silu)
                    a = actpool.tile([P, blk], bf16, name=f"a_{f}_{b}",
                                     tag=f"a{f}_{b}")
                    nc.vector.tensor_mul(a[:], sh[:], pg[:])
                    act_t[f][b] = a

            # ---- stage 2: y = act@W2 ----
            for d in range(dt):
                if d == 0:
                    w2_sb = w2_sb0
                else:
                    w2_sb = w2pool.tile([P, ft * P], bf16, name=f"w2_{d}",
                                        tag="w2")
                    nc.sync.dma_start(out=w2_sb[:], in_=w2_d[d])
                py0 = psy.tile([P, blk], f32, name="py0", tag="py0")
                py1 = psy.tile([P, blk], f32, name="py1", tag="py1")
                for f2 in range(ft):
                    w = w2_sb[:, f2 * P:(f2 + 1) * P]
                    nc.tensor.matmul(py0[:], w, act_t[f2][0][:],
                                     start=(f2 == 0), stop=(f2 == ft - 1))
                    nc.tensor.matmul(py1[:], w, act_t[f2][1][:],
                                     start=(f2 == 0), stop=(f2 == ft - 1))
                # evacuate the two PSUM tiles on different engines (DVE and
                # ACT) so the final block's copy isn't serialized at the tail
                for b, py in ((0, py0), (1, py1)):
                    y_sb = ypool.tile([P, blk], bf16, name=f"y{b}", tag="y")
                    if b == 0:
                        nc.vector.tensor_copy(y_sb[:], py[:])
                    else:
                        nc.scalar.activation(
                            y_sb[:], py[:], mybir.ActivationFunctionType.Copy)
                    nc.sync.dma_start(out=yt_d[d][:, b * blk:(b + 1) * blk],
                                      in_=y_sb[:])
    nc.compile()
    return nc


def _route(xt, Wr):
    """Replicate the reference's top-2 routing on host (fp32).

    Selection is robust: 2nd/3rd logit gaps are >> fp32 matmul noise.
    Stable argsort on -logits matches jax.lax.top_k tie-breaking
    (lower index first on exact ties).
    """
    logits = xt @ Wr                                     # [T, E] f32
    order = np.argsort(-logits, axis=1, kind="stable")[:, :2]
    v = np.take_along_axis(logits, order, axis=1)
    ex = np.exp(v - v[:, :1])
    probs = ex / ex.sum(axis=1, keepdims=True)           # [T, 2] f32
    return order, probs


def kernel(x, Wr, W1, W2, W3):
    global last_results
    x = np.asarray(x)
    Wr, W1, W2, W3 = (np.asarray(a) for a in (Wr, W1, W2, W3))
    b, s, D = x.shape
    E = Wr.shape[1]
    F = W1.shape[2]
    T = b * s
    assert E == N_CORES
    ko, ft, dt = D // P, F // P, D // P

    xt = np.ascontiguousarray(x.reshape(T, D), dtype=np.float32)
    order, probs = _route(xt, Wr)

    idx = [np.nonzero((order == e).any(axis=1))[0] for e in range(E)]
    maxc = max(len(i) for i in idx)
    assert maxc <= 1024

    # Token columns split into 2 blocks of width blk (mult of 4, <=512 for
    # one PSUM bank).
    blk = max(128, -(-maxc // 8) * 4)
    C = 2 * blk

    key = (ko, ft, dt, blk)
    if key not in _NC_CACHE:
        _NC_CACHE[key] = _build_nc(*key)
    nc = _NC_CACHE[key]

    bs = -(-blk // 32) * 32   # 64B-aligned block stride inside the x panel
    xq = xt.astype(BF16)
    in_maps = []
    for e in range(E):
        ids = idx[e]
        n = len(ids)
        xe = np.zeros((ko, P, 2 * bs), dtype=BF16)
        if n:
            xg = xq[ids].T.reshape(ko, P, n)
            n0 = min(n, blk)
            xe[:, :, :n0] = xg[:, :, :n0]
            if n > blk:
                xe[:, :, bs:bs + n - blk] = xg[:, :, blk:]
        # [P, ko*2*bs]: per-partition rows are contiguous, blocks 64B-aligned
        xe = np.ascontiguousarray(xe.transpose(1, 0, 2)).reshape(P, ko * 2 * bs)
        # [fg, P, 2*ko*P]: f-panel pairs grouped so each DMA has >=2KB rows
        w1t = np.ascontiguousarray(
            W1[e].astype(BF16).reshape(ko, P, ft // 2, 2, P)
            .transpose(2, 1, 3, 0, 4)
        ).reshape(ft // 2, P, 2 * ko * P)
        w3t = np.ascontiguousarray(
            W3[e].astype(BF16).reshape(ko, P, ft // 2, 2, P)
            .transpose(2, 1, 3, 0, 4)
        ).reshape(ft // 2, P, 2 * ko * P)
        w2t = np.ascontiguousarray(
            W2[e].astype(BF16).reshape(ft, P, dt, P).transpose(2, 1, 0, 3)
        ).reshape(dt, P, ft * P)
        in_maps.append({"xt": xe, "w1t": w1t, "w3t": w3t, "w2t": w2t})

    res = run_bass_kernel_spmd(nc, in_maps, core_ids=list(range(N_CORES)))
    last_results = res

    out = np.zeros((T, D), dtype=np.float64)
    for e in range(E):
        ids = idx[e]
        if not len(ids):
            continue
        ye = np.asarray(res.results[e]["yt"]).reshape(D, C)[:, :len(ids)]
        slot = (order[ids] == e).argmax(axis=1)
        pe = probs[ids, slot].astype(np.float64)
        out[ids] += ye.T.astype(np.float64) * pe[:, None]
    return out.astype(np.float32).reshape(b, s, D)
